# revision 14
# baseline (speedup 1.0000x reference)
import sys

sys.path.insert(0, "/opt/trn_rl_repo")

import numpy as np
from contextlib import ExitStack

# Problem constants (hardcoded per contract: kernel.py is self-contained).
B, S, D, O, M, E = 8, 2048, 768, 512, 1536, 8
T = S  # tokens per core (data-parallel over batch: 1 batch row per core)
P = 128
DT = D // P   # 6 d-tiles
MT = M // P   # 12 m-tiles
NT = T // P   # 16 token tiles per core
Q = 512       # token-quarter width (one PSUM bank of f32)
NQ = T // Q   # 4 quarters
NCORES = 8

_CACHE = {}


def _emit_body(nc, tile, tc, ctx, mybir, aps):
    """Emit one full forward pass. All inputs are pre-transposed bf16
    (except neg_cT, f32) so the device program is a pure GEMM pipeline."""
    from concourse.masks import make_identity

    f32 = mybir.dt.float32
    bf16 = mybir.dt.bfloat16
    AF = mybir.ActivationFunctionType
    ALU = mybir.AluOpType

    (xT_d, wg_d, negcT_d, winT_d, woutT_d, wscT_d, bo_d, out_d) = aps

    const = ctx.enter_context(tc.tile_pool(name="const", bufs=1))
    wt = ctx.enter_context(tc.tile_pool(name="wt", bufs=2))
    hp = ctx.enter_context(tc.tile_pool(name="hp", bufs=3))
    gwp = ctx.enter_context(tc.tile_pool(name="gw", bufs=2))
    pmm1 = ctx.enter_context(tc.tile_pool(name="pmm1", bufs=4, space="PSUM"))
    pmm2 = ctx.enter_context(tc.tile_pool(name="pmm2", bufs=2, space="PSUM"))
    ptr = ctx.enter_context(tc.tile_pool(name="ptr", bufs=2, space="PSUM"))

    # ---- persistent SBUF tensors ----
    xT = const.tile([P, DT, T], bf16)        # x^T: [d % 128, d // 128, t]
    acc = const.tile([P, NT, O], f32)        # output accumulator [t%128, t//128, o]
    wgate_sb = const.tile([P, DT, E], bf16)
    negcT = const.tile([P, MT, E], f32)      # -c transposed: [m%128, m//128, e]
    bo_sb = const.tile([P, O], bf16)         # b_out rows on first 8 partitions
    gexp = const.tile([P, T], f32)           # exp(logits)^T: [e, t] (8 rows)
    gbf = const.tile([P, T], bf16)
    g_exp = const.tile([P, NT, E], f32)      # exp(logits): [t%128, t//128, e]
    rinv = const.tile([P, NT], f32)          # 1 / sum_e exp
    gsum = const.tile([P, NT], f32)
    ident_f = const.tile([P, P], f32)

    make_identity(nc, ident_f)

    def load_expert(e):
        winT = wt.tile([P, DT, M], bf16, tag="winT")
        woutT = wt.tile([P, MT, O], bf16, tag="woutT")
        wscT = wt.tile([P, DT, O], bf16, tag="wscT")
        nc.sync.dma_start(winT, winT_d[e].rearrange("(dt p) m -> p dt m", p=P))
        nc.sync.dma_start(woutT, woutT_d[e].rearrange("(mt p) o -> p mt o", p=P))
        nc.sync.dma_start(wscT, wscT_d[e].rearrange("(dt p) o -> p dt o", p=P))
        return winT, woutT, wscT

    # ---- loads, ordered so PE can start ASAP: w_gate + x^T quarter 0
    # (gating q0 ramps the PE p-state), then expert-0 W_in (unblocks mm1 of
    # quarter 0), then the gelu bias, then the rest.
    nc.sync.dma_start(wgate_sb, wg_d.rearrange("(dt p) e -> p dt e", p=P))
    xT_src = xT_d.rearrange("(dt p) t -> p dt t", p=P)
    nc.sync.dma_start(xT[:, :, :Q], xT_src[:, :, :Q])
    win0 = wt.tile([P, DT, M], bf16, tag="winT")
    win0_src = winT_d[0].rearrange("(dt p) m -> p dt m", p=P)
    nc.sync.dma_start(win0[:, :, :M // 2], win0_src[:, :, :M // 2])
    nc.sync.dma_start(negcT, negcT_d.rearrange("(mt p) e -> p mt e", p=P))
    nc.sync.dma_start(win0[:, :, M // 2:], win0_src[:, :, M // 2:])
    nc.sync.dma_start(bo_sb[:E, :], bo_d)
    for q in range(1, NQ):
        nc.sync.dma_start(xT[:, :, q * Q:(q + 1) * Q],
                          xT_src[:, :, q * Q:(q + 1) * Q])
    wout0 = wt.tile([P, MT, O], bf16, tag="woutT")
    wsc0 = wt.tile([P, DT, O], bf16, tag="wscT")
    nc.sync.dma_start(wout0, woutT_d[0].rearrange("(mt p) o -> p mt o", p=P))
    nc.sync.dma_start(wsc0, wscT_d[0].rearrange("(dt p) o -> p dt o", p=P))
    cur = (win0, wout0, wsc0)

    # ---- gating: gexp[e, t] = exp(x @ w_gate)^T, then transpose + rowsum.
    # Quarter 0 is emitted first (ramps the PE p-state while expert-0 W_in
    # streams in); the rest is emitted after mm1(e0, q0) below — none of it
    # is needed until the first gate-weighted accumulate.
    def emit_gate_q(q):
        pg = ptr.tile([P, Q], f32, tag="gate")
        for dt_ in range(DT):
            nc.tensor.matmul(pg[:E, :], wgate_sb[:, dt_, :],
                             xT[:, dt_, q * Q:(q + 1) * Q],
                             start=(dt_ == 0), stop=(dt_ == DT - 1))
        nc.scalar.activation(gexp[:E, q * Q:(q + 1) * Q], pg[:E, :], AF.Exp)

    def emit_gate_rest():
        for q in range(1, NQ):
            emit_gate_q(q)
        nc.vector.tensor_copy(gbf[:E, :], gexp[:E, :])

        # g_exp[t, e] via PE transposes of 128-token blocks
        for tt in range(NT):
            pt = ptr.tile([P, Q], f32, tag="gate")
            nc.tensor.transpose(pt[:, :E], gexp[:E, tt * P:(tt + 1) * P],
                                ident_f[:E, :E])
            nc.vector.tensor_copy(g_exp[:, tt, :], pt[:, :E])

        nc.vector.tensor_reduce(gsum, g_exp, axis=mybir.AxisListType.X,
                                op=ALU.add)
        nc.vector.reciprocal(rinv, gsum)

        # acc init: acc[t, o] = (g[t, :] @ b_out) * rinv[t]
        for tt in range(NT):
            pb = pmm2.tile([P, O], f32, tag="mm2")
            nc.tensor.matmul(pb, gbf[:E, tt * P:(tt + 1) * P], bo_sb[:E, :])
            nc.vector.tensor_scalar_mul(acc[:, tt, :], pb,
                                        scalar1=rinv[:, tt:tt + 1])

    emit_gate_q(0)

    # ---- expert loop: 32 (expert, quarter) units, one-quarter PE lookahead ----
    def emit_mm1(e, q, h, winT):
        for mt in range(MT):
            ph = pmm1.tile([P, Q], f32, tag="mm1")
            for dt_ in range(DT):
                nc.tensor.matmul(ph, winT[:, dt_, mt * P:(mt + 1) * P],
                                 xT[:, dt_, q * Q:(q + 1) * Q],
                                 start=(dt_ == 0), stop=(dt_ == DT - 1))
            nc.scalar.activation(h[:, mt, :], ph, AF.Gelu,
                                 bias=negcT[:, mt, e:e + 1], scale=1.0)

    def emit_gw(e):
        # gw[t] = g_exp[t, e] * rinv[t]: the per-token weight for expert e,
        # premultiplied so the accumulate below is one fused DVE op.
        gw = gwp.tile([P, NT], f32, tag="gw")
        nc.vector.scalar_tensor_tensor(out=gw, in0=g_exp[:, :, e], scalar=1.0,
                                       in1=rinv, op0=ALU.mult, op1=ALU.mult)
        return gw

    def emit_mm2(e, q, h, woutT, wscT, gw):
        for t8 in range(Q // P):
            tg = q * (Q // P) + t8
            po = pmm2.tile([P, O], f32, tag="mm2")
            for mt in range(MT):
                nc.tensor.matmul(po, h[:, mt, t8 * P:(t8 + 1) * P],
                                 woutT[:, mt, :], start=(mt == 0), stop=False)
            for dt_ in range(DT):
                nc.tensor.matmul(po, xT[:, dt_, tg * P:(tg + 1) * P],
                                 wscT[:, dt_, :], start=False,
                                 stop=(dt_ == DT - 1))
            nc.vector.scalar_tensor_tensor(out=acc[:, tg, :], in0=po,
                                           scalar=gw[:, tg:tg + 1],
                                           in1=acc[:, tg, :],
                                           op0=ALU.mult, op1=ALU.add)
            if e == E - 1:
                nc.sync.dma_start(out_d[tg * P:(tg + 1) * P, :], acc[:, tg, :])

    # mm1 of (e0, q0) right after gate-q0, then the rest of gating/init
    # (fills the PE while expert-0's W_out/W_sc and x quarters stream in).
    h0 = hp.tile([P, MT, Q], bf16, tag="h")
    emit_mm1(0, 0, h0, cur[0])
    emit_gate_rest()
    cur_gw = emit_gw(0)
    prev = (0, 0, h0, cur[1], cur[2], cur_gw)

    for e in range(E):
        if e + 1 < E:
            nxt = load_expert(e + 1)
        if e > 0:
            cur_gw = emit_gw(e)
        for q in range(NQ):
            if e == 0 and q == 0:
                continue
            h = hp.tile([P, MT, Q], bf16, tag="h")
            emit_mm1(e, q, h, cur[0])
            emit_mm2(*prev)
            prev = (e, q, h, cur[1], cur[2], cur_gw)
        if e + 1 < E:
            cur = nxt
    emit_mm2(*prev)


def _build(reps=1):
    import concourse.bass as bass
    import concourse.tile as tile
    from concourse import bacc, mybir

    f32 = mybir.dt.float32
    bf16 = mybir.dt.bfloat16

    nc = bacc.Bacc("TRN2", target_bir_lowering=False, debug=False,
                   num_devices=NCORES)

    xT_d = nc.dram_tensor("xT", (D, T), bf16, kind="ExternalInput").ap()
    wg_d = nc.dram_tensor("w_gate", (D, E), bf16, kind="ExternalInput").ap()
    negcT_d = nc.dram_tensor("neg_cT", (M, E), f32, kind="ExternalInput").ap()
    winT_d = nc.dram_tensor("W_inT", (E, D, M), bf16, kind="ExternalInput").ap()
    woutT_d = nc.dram_tensor("W_outT", (E, M, O), bf16,
                             kind="ExternalInput").ap()
    wscT_d = nc.dram_tensor("W_scT", (E, D, O), bf16, kind="ExternalInput").ap()
    bo_d = nc.dram_tensor("b_out", (E, O), bf16, kind="ExternalInput").ap()
    out_d = nc.dram_tensor("out", (T, O), f32, kind="ExternalOutput").ap()
    aps = (xT_d, wg_d, negcT_d, winT_d, woutT_d, wscT_d, bo_d, out_d)

    with tile.TileContext(nc) as tc:
        # reps > 1 unrolls the whole body back-to-back; used only by the
        # timing harness (loop-differencing cancels the dispatch constant).
        for _ in range(reps):
            with ExitStack() as ctx:
                _emit_body(nc, tile, tc, ctx, mybir, aps)

    nc.compile()
    return nc


def _get_nc(reps=1):
    key = ("nc", reps)
    if key not in _CACHE:
        _CACHE[key] = _build(reps)
    return _CACHE[key]


def prepare_shared(w_gate, bias_in, W_in, W_out, b_out, W_sc):
    """Host-side layout prep: bf16 casts + contraction-major transposes.
    Arithmetic on device is identical to casting on-chip (as the original
    kernel did); only the layout work moves to the host."""
    import ml_dtypes
    bf16 = ml_dtypes.bfloat16
    W_in = np.asarray(W_in, np.float32)
    neg_cT = -np.einsum("ed,emd->me", np.asarray(bias_in, np.float64),
                        np.asarray(W_in, np.float64)).astype(np.float32)
    return {
        "w_gate": np.ascontiguousarray(np.asarray(w_gate, np.float32)).astype(bf16),
        "neg_cT": np.ascontiguousarray(neg_cT),
        "W_inT": np.ascontiguousarray(
            W_in.transpose(0, 2, 1)).astype(bf16),               # (E, D, M)
        "W_outT": np.ascontiguousarray(
            np.asarray(W_out, np.float32).transpose(0, 2, 1)).astype(bf16),
        "W_scT": np.ascontiguousarray(
            np.asarray(W_sc, np.float32).transpose(0, 2, 1)).astype(bf16),
        "b_out": np.ascontiguousarray(np.asarray(b_out, np.float32)).astype(bf16),
    }


def prepare_xT(x_core):
    import ml_dtypes
    return np.ascontiguousarray(
        np.asarray(x_core, np.float32).T).astype(ml_dtypes.bfloat16)


def kernel(x, w_gate, bias_in, W_in, W_out, b_out, W_sc):
    from concourse.bass_utils import run_bass_kernel_spmd

    nc = _get_nc()
    shared = prepare_shared(w_gate, bias_in, W_in, W_out, b_out, W_sc)
    x = np.asarray(x, np.float32)
    in_maps = [{"xT": prepare_xT(x[i]), **shared} for i in range(NCORES)]
    res = run_bass_kernel_spmd(nc, in_maps, core_ids=list(range(NCORES)))
    out = np.stack([res.results[i]["out"] for i in range(NCORES)], axis=0)
    return out.astype(np.float32)
